# revision 28
# baseline (speedup 1.0000x reference)
"""Trainium2 Bass kernel: per-pixel channel shuffle + 3x3 conv (stride 1, pad 1).

Problem: x [32,256,56,56] f32, w [256,256,3,3] f32 (OIHW), perm [3136,256] i32;
out[b,:,h,w] = conv3x3(xs)[b,:,h,w] where xs[b,:,l] = x[b, perm[l,:], l].

Strategy (8 NeuronCores, data-parallel over batch, 4 batches/core):
  host: x -> pixel-major bf16 packed [B_LOC, 7, 112, 1024] (448 px per scatter
        call, 4 pixels per partition); inverse-perm int16 tables in the same
        layout; w pre-transposed into a [128, 36*128] lhsT sheet.
  device, per batch:
    1. 7 contiguous DMAs of [112, 1024] call tiles (448 px = 8 image rows).
    2. 7 GPSIMD local_scatter calls (channels=112, num_idxs=1024) apply each
       pixel's inverse channel permutation within its partition.
    3. Per call: 8 PE transposes [112,128]->[128,112] produce [c, l] 2-row
       strips in one PSUM bank; 2 strided DVE copies place all 8 rows into a
       zero-padded 58-wide flat image.
    4. Conv as implicit GEMM, weight-stationary: 8 groups x 406 px (7 rows),
       2 passes of 4 PSUM banks; each of the pass's 18 lhsT loads streams all
       4 groups back-to-back, hiding LDWEIGHTS. Scalar strips the padding
       while evicting PSUM to bf16 and triggers the output DMAs so they never
       block input DMAs on the sync queue; host upcasts to f32.
  Schedule: batch 0 chases the scatter pipeline group-by-group (group g only
  needs scatter calls through ceil((7g+8)/8)); batches 1-3 weave their shuffle
  calls between the previous batch's conv chunks so GPSIMD runs a full batch
  ahead and the PE never stalls. Head DMAs (call-0/1 inputs, split in half,
  then weights split 8 ways on the scalar queue) are ordered for first-scatter
  latency. Measured ~243.5us on TRN2 vs 195us conv-stream floor (PE busy ~88%,
  remaining idle = batch-0 fill + fixed pre/postamble).
"""

import os
import sys
import types
import numpy as np

_STATE = {}
LAST_RESULT = None

B, C, H, W = 32, 256, 56, 56
HW = H * W          # 3136
PADW = 58
XS_LEN = 3376       # 58 rows x 58 cols + 12 slack
N_CORES = 8
B_LOC = B // N_CORES  # 4

NCALL = 7           # scatter calls per batch
PPC = 448           # pixels per call (8 image rows)
PPP = 4             # pixels per partition per call
CH = 112            # partitions (channels arg) per scatter call
NIDX = PPP * C      # 1024 idx / elems per partition per call

NG = 8              # conv groups per oct
GROUP = 406         # group stride in the padded image (7 rows x 58)
NWIN = 392          # matmul stream length: 7 rows x 56 real cols (pads skipped)
GROWS = 7           # image rows per group
PASSES = ((0, 1, 2, 3), (4, 5, 6, 7))


def _install_ntff_shim():
    # antenv.axon_hooks is absent in some images; provide it so trace=True
    # (BASS_TRACE=1) can capture NTFF profiles instead of crashing.
    name = "antenv.axon_hooks"
    if name in sys.modules:
        return
    try:
        import antenv  # noqa: F401

        m = types.ModuleType(name)
        m._hook = None
        m.set_axon_ntff_profile_hook = lambda h: setattr(m, "_hook", h)
        m.get_axon_ntff_profile_hook = lambda: m._hook
        sys.modules[name] = m
        setattr(sys.modules["antenv"], "axon_hooks", m)
        from trn_agent_boot.trn_boot import _ntff_profile_via_ctypes

        hook = _ntff_profile_via_ctypes("/opt/axon/libaxon_pjrt.so")
        if hook is not None:
            m.set_axon_ntff_profile_hook(hook)
    except Exception:
        pass


def _build_kernel():
    import concourse.bass as bass
    import concourse.mybir as mybir
    from concourse import bacc, tile
    from concourse.masks import make_identity
    from contextlib import ExitStack

    F32 = mybir.dt.float32
    BF16 = mybir.dt.bfloat16
    I16 = mybir.dt.int16

    nc = bacc.Bacc("TRN2", target_bir_lowering=False, debug=False, num_devices=N_CORES)

    xb = nc.dram_tensor("xb", [B_LOC, NCALL, CH, NIDX], BF16, kind="ExternalInput")
    wt = nc.dram_tensor("wt", [128, 36 * 128], BF16, kind="ExternalInput")
    idxt = nc.dram_tensor("idxt", [CH, NCALL * NIDX], I16, kind="ExternalInput")
    out = nc.dram_tensor("out", [B_LOC, C, HW], BF16, kind="ExternalOutput")

    with tile.TileContext(nc) as tc, ExitStack() as ctx:
        const = ctx.enter_context(tc.tile_pool(name="const", bufs=1))
        xin_pool = ctx.enter_context(tc.tile_pool(name="xin", bufs=10))
        sout_pool = ctx.enter_context(tc.tile_pool(name="sout", bufs=10))

        def xin_dma(b, k, nsplit=1, eng="sync"):
            xin = xin_pool.tile([128, NIDX], BF16, name="xin", tag="xin")
            step = NIDX // nsplit
            for ci, c0 in enumerate(range(0, NIDX, step)):
                e = eng if eng != "both" else ("sync" if ci % 2 == 0 else "scalar")
                getattr(nc, e).dma_start(
                    out=xin[0:CH, c0 : c0 + step], in_=xb[b, k, :, c0 : c0 + step]
                )
            return xin

        wsb = const.tile([128, 36 * 128], BF16)
        idxtiles = {}

        def idx_dma(k, nsplit=1, eng="sync"):
            t = const.tile([128, NIDX], I16, name=f"idx{k}", tag=f"idx{k}")
            idxtiles[k] = t
            step = NIDX // nsplit
            for ci, c0 in enumerate(range(0, NIDX, step)):
                e = eng if eng != "both" else ("sync" if ci % 2 == 0 else "scalar")
                getattr(nc, e).dma_start(
                    out=t[0:CH, c0 : c0 + step],
                    in_=idxt[:, k * NIDX + c0 : k * NIDX + c0 + step],
                )

        # critical-path prefetch: call-0 inputs split across engines, then
        # call-1, then weights, then the rest.
        xin_pre = {}
        # head critical path: first two calls' inputs on sync, weights on the
        # scalar queue, bulk idx tables behind.
        idx_dma(0, nsplit=2)
        xin_pre[(0, 0)] = xin_dma(0, 0, nsplit=2)
        idx_dma(1, nsplit=2)
        xin_pre[(0, 1)] = xin_dma(0, 1, nsplit=2)
        for q in range(8):
            nc.scalar.dma_start(
                out=wsb[:, q * 576 : (q + 1) * 576],
                in_=wt[:, q * 576 : (q + 1) * 576],
            )
        ident = const.tile([128, 128], BF16)
        make_identity(nc, ident[:, :])
        xs_pool = ctx.enter_context(tc.tile_pool(name="xs", bufs=2))
        ost_pool = ctx.enter_context(tc.tile_pool(name="ost", bufs=4))
        tps_pool = ctx.enter_context(tc.tile_pool(name="tps", bufs=2, space="PSUM"))
        mps_pool = ctx.enter_context(tc.tile_pool(name="mps", bufs=6, space="PSUM"))

        xs_tiles = {}

        def shuffle_call(b, k):
            # DMA a 448-pixel tile in [partition=pixel%112, (j, c)] layout,
            # scatter channels within each partition, transpose back to [c, l]
            # and place the 2-row strips into the padded image.
            if k == 0:
                xs = xs_pool.tile([128, 2 * XS_LEN], BF16, name="xs", tag="xs")
                xs_tiles[b] = xs
                for ct in range(2):
                    base = ct * XS_LEN
                    nc.vector.memset(xs[:, base : base + PADW], 0.0)
                    nc.vector.memset(xs[:, base + 57 * PADW : base + XS_LEN], 0.0)
                    nc.vector.memset(
                        xs[:, base + PADW : base + PADW + 56 * PADW].rearrange(
                            "p (r x) -> p r x", r=56
                        )[:, :, 0:1],
                        0.0,
                    )
                    nc.vector.memset(
                        xs[
                            :, base + PADW + 57 : base + PADW + 57 + 56 * PADW
                        ].rearrange("p (r x) -> p r x", r=56)[:, :, 0:1],
                        0.0,
                    )
            xs = xs_tiles[b]

            if k not in idxtiles:
                idx_dma(k)
            xin = xin_pre.pop((b, k), None)
            if xin is None:
                xin = xin_dma(b, k)
            sout = sout_pool.tile([128, NIDX], BF16, name="sout", tag="sout")
            nc.gpsimd.local_scatter(
                out_ap=sout[0:CH, :],
                data_ap=xin[0:CH, :],
                idxs_ap=idxtiles[k][0:CH, :],
                channels=CH,
                num_elems=NIDX,
                num_idxs=NIDX,
            )
            tps = tps_pool.tile([128, 8 * CH], BF16, name="tps", tag="tps")
            for j in range(PPP):
                for ct in range(2):
                    nc.tensor.transpose(
                        tps[:, ct * 448 + j * CH : ct * 448 + (j + 1) * CH],
                        sout[0:CH, j * C + ct * 128 : j * C + ct * 128 + 128],
                        ident[0:CH, 0:CH],
                    )
            q0 = 59 + 8 * k * PADW
            for ct in range(2):
                nc.vector.tensor_copy(
                    xs[:, ct * XS_LEN + q0 : ct * XS_LEN + q0 + 8 * PADW].rearrange(
                        "p (j r x) -> p j r x", j=PPP, r=2
                    )[:, :, :, 0:56],
                    tps[:, ct * 448 : ct * 448 + 448].rearrange(
                        "p (j r x) -> p j r x", j=PPP, r=2
                    ),
                )

        def conv_chunk(b, oct, groups):
            # weight-stationary: each of the 18 lhsT tiles streams all groups
            # of this pass before the next weight load.
            xs = xs_tiles[b]
            mps = {
                g: mps_pool.tile([128, NWIN], F32, name="mp", tag="mp")
                for g in groups
            }
            for i in range(18):
                ct, tap = divmod(i, 9)
                dh, dw = divmod(tap, 3)
                delta = (dh - 1) * PADW + (dw - 1)
                widx = (ct * 9 + tap) * 2 + oct
                for g in groups:
                    q0 = 59 + g * GROUP + delta
                    # stream only the 56 real cols of each of the 7 rows
                    nc.tensor.matmul(
                        mps[g][:, :],
                        lhsT=wsb[:, widx * 128 : (widx + 1) * 128],
                        rhs=xs[
                            :, ct * XS_LEN + q0 : ct * XS_LEN + q0 + GROUP
                        ].rearrange("p (r x) -> p r x", r=GROWS)[:, :, 0:56],
                        start=(i == 0),
                        stop=(i == 17),
                    )
            gw = GROWS * W
            for gp in range(0, len(groups), 2):
                pair = groups[gp : gp + 2]
                ost = ost_pool.tile([128, 2 * gw], BF16, name="ost", tag="ost")
                for idx, g in enumerate(pair):
                    nc.scalar.copy(ost[:, idx * gw : (idx + 1) * gw], mps[g][:, :])
                g0 = pair[0]
                nc.scalar.dma_start(
                    out=out[
                        b,
                        oct * 128 : (oct + 1) * 128,
                        g0 * gw : g0 * gw + len(pair) * gw,
                    ],
                    in_=ost[:, 0 : len(pair) * gw],
                )

        # Weave: batch b's conv chunks interleave with batch b+1's shuffle
        # calls so GPSIMD runs a full batch ahead of PE.
        # Batch 0 special-cases an early start: pass A only needs calls 0-3.
        shuffle_call(0, 0)
        conv_chunk(0, 0, (0,))
        conv_chunk(0, 1, (0,))
        shuffle_call(0, 1)
        conv_chunk(0, 0, (1,))
        conv_chunk(0, 1, (1,))
        shuffle_call(0, 2)
        shuffle_call(0, 3)
        conv_chunk(0, 0, (2, 3))
        conv_chunk(0, 1, (2, 3))
        shuffle_call(0, 4)
        shuffle_call(0, 5)
        conv_chunk(0, 0, (4, 5))
        conv_chunk(0, 1, (4, 5))
        shuffle_call(0, 6)
        conv_chunk(0, 0, (6, 7))
        conv_chunk(0, 1, (6, 7))

        for b in range(1, B_LOC):
            for k in range(4):
                shuffle_call(b, k)
            conv_chunk(b, 0, PASSES[0])
            shuffle_call(b, 4)
            shuffle_call(b, 5)
            conv_chunk(b, 1, PASSES[0])
            shuffle_call(b, 6)
            if b == B_LOC - 1:
                conv_chunk(b, 0, PASSES[1])
                conv_chunk(b, 1, (4, 5))
                conv_chunk(b, 1, (6, 7))
            else:
                conv_chunk(b, 0, PASSES[1])
                conv_chunk(b, 1, PASSES[1])

    nc.compile()
    return nc


def _host_prep(x, w, perm):
    import ml_dtypes

    # pixel-major bf16 x packed per scatter call: [B, NCALL, CH, PPP*C]
    xf = (
        x.transpose(0, 2, 3, 1)
        .reshape(B, NCALL, PPP, CH, C)
        .transpose(0, 1, 3, 2, 4)
        .reshape(B, NCALL, CH, NIDX)
        .astype(ml_dtypes.bfloat16)
    )

    wtile = np.empty((36, 128, 128), dtype=ml_dtypes.bfloat16)
    wf = np.asarray(w, dtype=np.float32)
    for ct in range(2):
        for tap in range(9):
            kh, kw = divmod(tap, 3)
            for oct in range(2):
                i = (ct * 9 + tap) * 2 + oct
                wtile[i] = wf[
                    oct * 128 : (oct + 1) * 128, ct * 128 : (ct + 1) * 128, kh, kw
                ].T.astype(ml_dtypes.bfloat16)
    wtile = np.ascontiguousarray(wtile.transpose(1, 0, 2).reshape(128, 36 * 128))

    # inverse permutation: iperm[l, c] = position of channel c in xs at pixel l
    iperm = np.empty((HW, C), dtype=np.int16)
    np.put_along_axis(
        iperm, perm.astype(np.int64), np.arange(C, dtype=np.int16)[None, :], axis=1
    )
    # scatter idx table: idxt[p, k*NIDX + j*C + c] = j*C + iperm[448k+112j+p, c]
    ip = iperm.reshape(NCALL, PPP, CH, C).transpose(2, 0, 1, 3).copy()
    ip += (np.arange(PPP, dtype=np.int16) * C)[None, None, :, None]
    idxt = np.ascontiguousarray(ip.reshape(CH, NCALL * NIDX))

    in_maps = []
    for cidx in range(N_CORES):
        in_maps.append(
            {
                "xb": np.ascontiguousarray(xf[cidx * B_LOC : (cidx + 1) * B_LOC]),
                "wt": wtile,
                "idxt": idxt,
            }
        )
    return in_maps


def kernel(x, w, perm):
    global LAST_RESULT
    _install_ntff_shim()
    from concourse.bass_utils import run_bass_kernel_spmd

    x = np.asarray(x, dtype=np.float32)
    w = np.asarray(w, dtype=np.float32)
    perm = np.asarray(perm)

    if "nc" not in _STATE:
        _STATE["nc"] = _build_kernel()
    nc = _STATE["nc"]

    in_maps = _host_prep(x, w, perm)
    res = run_bass_kernel_spmd(nc, in_maps, core_ids=list(range(N_CORES)))
    LAST_RESULT = res
    out = np.concatenate(
        [r["out"].reshape(B_LOC, C, H, W) for r in res.results], axis=0
    )
    return out.astype(np.float32)
